# revision 49
# baseline (speedup 1.0000x reference)
"""Bidirectional ConvLSTM + 1x1 proj + BatchNorm + ReLU + skip, on 8 trn2 cores.

Sharding: data-parallel over batch (B=8 -> 1 batch element per core).
BatchNorm batch statistics are reduced across cores with a tiny AllReduce.

Per-core layout: channels on SBUF partitions (96), W on the free dim.
The H-recurrence runs forward and backward interleaved (192 slots x 2 dirs).
Each step's gate conv = 24 bf16 matmuls (4 gate blocks x 3 taps x {x,h}),
K=96(+1 bias row for x), M=96, N=192, accumulated in PSUM.
Hidden states are written (bf16) into a width-padded SBUF store that serves
as both the recurrence input (taps read the zero pad columns) and the
projection input.

Pass A (in-scan, lagged) computes y = proj(h_f, h_b) for completed rows,
feeds bn_stats, and writes y (bf16) back into the dead hs slots.  Pass B
after the stats AllReduce is then matmul-free: relu(a*y+b) + x -> out,
with x prefetched on the sync DMA queue during the AllReduce.
"""

import os
import sys
import types

import numpy as np
import ml_dtypes

B, C, H, W = 8, 96, 192, 192
HC = 96
EPS = 1e-5
NCORES = 8
WP = W + 2          # padded row width in the hidden-state store
NSLOT = H + 1       # store slots (one zero slot for the initial state)
HWTOT = H * W       # 36864
NCHUNK = H // 2     # projection chunks (2 rows each, N=384)

_cached = {}


def _install_ntff_hook():
    # Optional: lets BASS_TRACE=1 produce an NTFF profile under axon.
    if 'antenv.axon_hooks' in sys.modules:
        return
    try:
        import trn_agent_boot.trn_boot as tb
        hook = tb._ntff_profile_via_ctypes('/opt/axon/libaxon_pjrt.so')
        mod = types.ModuleType('antenv.axon_hooks')
        mod.get_axon_ntff_profile_hook = lambda: hook
        mod.set_axon_ntff_profile_hook = lambda h: None
        sys.modules['antenv.axon_hooks'] = mod
    except Exception:
        pass


def _prep_weights(w_f, b_f, w_b, b_b, w_proj):
    """Host-side weight packing into matmul-friendly lhsT layouts (bf16).

    Gate blocks (psum column order): 0=i, 1=f, 2=o, 3=g.
    Reference gate row ranges: i=0:96, f=96:192, g=192:288, o=288:384.
    wx[d]: [97, 12, 96]  (k=channel(+bias row 96), j=gb*3+tap, m=gate unit)
    wh[d]: [96, 12, 96]
    wp:    [96, 2, 96]   (k=hidden ch, d, m=out ch)
    """
    bf16 = ml_dtypes.bfloat16
    # gate block order: f, g, i, o  (bank0 = [f|g], bank1 = [i|o])
    gate_rows = [slice(96, 192), slice(192, 288), slice(0, 96), slice(288, 384)]
    wx = np.zeros((2, 97, 12, 96), np.float32)
    wh = np.zeros((2, 96, 12, 96), np.float32)
    for d, (w4, bias) in enumerate(((w_f, b_f), (w_b, b_b))):
        wmid = w4[:, :, 1, :]          # [384, 192, 3]
        for gb in range(4):
            rows = gate_rows[gb]
            for tap in range(3):
                j = gb * 3 + tap
                # x part: channels 0:96
                wx[d, 0:96, j, :] = wmid[rows, 0:96, tap].T
                # h part: channels 96:192
                wh[d, :, j, :] = wmid[rows, 96:192, tap].T
            # bias on the center tap's extra ones-row
            wx[d, 96, gb * 3 + 1, :] = bias[rows]
    wp = np.zeros((96, 2, 96), np.float32)
    wp[:, 0, :] = w_proj[:, 0:96].T
    wp[:, 1, :] = w_proj[:, 96:192].T
    return wx.astype(bf16), wh.astype(bf16), wp.astype(bf16)


def _build_program():
    import concourse.bass as bass
    import concourse.bacc as bacc
    import concourse.tile as tile
    from concourse import mybir

    f32 = mybir.dt.float32
    bf16 = mybir.dt.bfloat16
    AF = mybir.ActivationFunctionType

    nc = bacc.Bacc('TRN2', target_bir_lowering=False, debug=False,
                   num_devices=NCORES)

    x_d = nc.dram_tensor("x", [C, HWTOT], f32, kind="ExternalInput")
    xbf_d = nc.dram_tensor("xbf", [C, HWTOT], bf16, kind="ExternalInput")
    wx_d = nc.dram_tensor("wx", [97, 2, 12, 96], bf16, kind="ExternalInput")
    wh_d = nc.dram_tensor("wh", [96, 2, 12, 96], bf16, kind="ExternalInput")
    wp_d = nc.dram_tensor("wp", [96, 2, 96], bf16, kind="ExternalInput")
    gb_d = nc.dram_tensor("gamma_beta", [96, 2], f32, kind="ExternalInput")
    out_d = nc.dram_tensor("out", [C, HWTOT], bf16, kind="ExternalOutput")

    with tile.TileContext(nc) as tc:
        with (
            tc.tile_pool(name="const", bufs=1) as const,
            tc.tile_pool(name="dram", bufs=1, space="DRAM") as dram,
        ):
            # --- constants / persistent state -------------------------------
            gb_s = const.tile([96, 2], f32, name="gb_s")
            nc.gpsimd.dma_start(gb_s[:], gb_d[:])
            eps_s = const.tile([96, 1], f32, name="eps_s")
            nc.vector.memset(eps_s[:], EPS)

            # hidden-state stores, slot r width-padded. Only the pad columns
            # and the zero-state slot need zeroing (the scan writes the rest).
            hs = []
            for d in range(2):
                st = const.tile([96, NSLOT, WP], bf16, name=f"hs{d}")
                nc.vector.memset(st[:, :, 0:1], 0.0)
                nc.vector.memset(st[:, :, WP - 1:WP], 0.0)
                zslot = 0 if d == 0 else NSLOT - 1
                nc.vector.memset(st[:, zslot, :], 0.0)
                hs.append(st)

            statb = const.tile([96, NCHUNK, 6], f32, name="statb")
            mv_a = const.tile([96, 2], f32, name="mv_a")
            mv_b = const.tile([96, 2], f32, name="mv_b")

            # --- the scan ---------------------------------------------------
            with (
                tc.tile_pool(name="scanc", bufs=1) as scanc,
                tc.tile_pool(name="sact", bufs=4) as sact,
                tc.tile_pool(name="tmp", bufs=6) as tmp,
                tc.tile_pool(name="gpsum", bufs=2, space="PSUM") as gpsum,
            ):
                # scan-lifetime constants: freed before pass B so its DMA
                # pipeline can buffer deeper
                wx_s = scanc.tile([97, 2, 12, 96], bf16, name="wx_s")
                nc.gpsimd.dma_start(wx_s[:], wx_d[:])
                wh_s = scanc.tile([96, 2, 12, 96], bf16, name="wh_s")
                nc.gpsimd.dma_start(wh_s[:], wh_d[:])
                wp_s = scanc.tile([96, 2, 96], bf16, name="wp_s")
                nc.gpsimd.dma_start(wp_s[:], wp_d[:])

                # x-row tiles (97 rows: 96 ch + ones row), 4-deep rotation per
                # dir so the row DMA prefetches ahead of the matmul reads
                XRDEPTH = 4
                xr = [[None] * XRDEPTH, [None] * XRDEPTH]
                for d in range(2):
                    for p in range(XRDEPTH):
                        t = scanc.tile([97, WP], bf16, name=f"xr{d}{p}")
                        nc.vector.memset(t[0:96, 0:1], 0.0)
                        nc.vector.memset(t[0:96, WP - 1:WP], 0.0)
                        nc.vector.memset(t[96:97, :], 1.0)
                        xr[d][p] = t

                # cell state fp32, ping-pong per dir
                ctl = [[None, None], [None, None]]
                for d in range(2):
                    for p in range(2):
                        t = scanc.tile([96, W], f32, name=f"c{d}{p}")
                        nc.vector.memset(t[:], 0.0)
                        ctl[d][p] = t
                def passA(k, ra, rb):
                    """proj for rows (ra, rb) -> bn_stats; y (bf16) back into
                    the now-dead hs slots: y[ra] -> hs0 slot ra+1, y[rb] ->
                    hs1 slot rb."""
                    psA = gpsum.tile([96, 512], f32, name="psA")
                    nc.tensor.matmul(psA[:, 0:192], wp_s[:, 0, :],
                                     hs[0][:, rb + 1, 1:1 + W],
                                     start=True, stop=False)
                    nc.tensor.matmul(psA[:, 0:192], wp_s[:, 1, :],
                                     hs[1][:, rb, 1:1 + W],
                                     start=False, stop=False)
                    nc.tensor.matmul(psA[:, 192:384], wp_s[:, 0, :],
                                     hs[0][:, ra + 1, 1:1 + W],
                                     start=False, stop=False)
                    nc.tensor.matmul(psA[:, 192:384], wp_s[:, 1, :],
                                     hs[1][:, ra, 1:1 + W],
                                     start=False, stop=True)
                    nc.vector.bn_stats(statb[:, k, :], psA[:, 0:384])
                    nc.vector.tensor_copy(hs[1][:, rb, 1:1 + W],
                                          psA[:, 0:192])
                    nc.vector.tensor_copy(hs[0][:, ra + 1, 1:1 + W],
                                          psA[:, 192:384])

                # prime the x-row tiles for step 0
                for d in range(2):
                    r0 = 0 if d == 0 else H - 1
                    nc.gpsimd.dma_start(xr[d][0][0:96, 1:1 + W],
                                        xbf_d[:, r0 * W:(r0 + 1) * W])

                for t in range(H):
                    p, pn = t & 1, (t + 1) & 1
                    px = t % XRDEPTH

                    # prefetch next step's x rows before this step's t1 ops
                    # can block the gpsimd queue
                    if t + 1 < H:
                        pxn = (t + 1) % XRDEPTH
                        for d in range(2):
                            rn = t + 1 if d == 0 else H - 2 - t
                            nc.gpsimd.dma_start(xr[d][pxn][0:96, 1:1 + W],
                                                xbf_d[:, rn * W:(rn + 1) * W])

                    # fused pass A, lagged 2 slots so it never waits on the
                    # current slot's h2: project rows completed at slot t-2.
                    if t >= H // 2 + 2:
                        passA(t - H // 2 - 2, t - 2, H - 1 - (t - 2))
                    if t == 152:
                        # first-half stats aggregate, hidden in the scan
                        nc.vector.bn_aggr(mv_a[:], statb[:, 0:NCHUNK // 2, :])

                    for d in range(2):
                        r = t if d == 0 else H - 1 - t
                        # prev-h slot / out slot in the store
                        sl_in = r if d == 0 else r + 1
                        sl_out = r + 1 if d == 0 else r

                        # bank0 = [f|g], bank1 = [i|o]; bank0 first so that
                        # sig(f)/tanh(g)/t1 run under bank1's matmuls and the
                        # post-group chain is just sig(i,o)->t2->c2->tanh->h2.
                        ps0 = gpsum.tile([96, 512], f32, name=f"ps0_{d}", bufs=1)
                        ps1 = gpsum.tile([96, 512], f32, name=f"ps1_{d}")
                        hin = hs[d][:, sl_in, :]
                        banks = ((ps0, 0), (ps0, 192), (ps1, 0), (ps1, 192))
                        # all 12 x-matmuls first: they don't depend on h2, so
                        # the prior step's gate tail hides under them
                        for gbi, (pst, col) in enumerate(banks):
                            for tap in range(3):
                                j = gbi * 3 + tap
                                nc.tensor.matmul(
                                    pst[:, col:col + 192],
                                    wx_s[:, d, j, :],
                                    xr[d][px][:, tap:tap + W],
                                    start=(col == 0 and tap == 0),
                                    stop=False,
                                )
                        # then the 12 h-matmuls
                        for gbi, (pst, col) in enumerate(banks):
                            for tap in range(3):
                                j = gbi * 3 + tap
                                nc.tensor.matmul(
                                    pst[:, col:col + 192],
                                    wh_s[:, d, j, :],
                                    hin[:, tap:tap + W],
                                    start=False,
                                    stop=(col == 192 and tap == 2),
                                )

                            if gbi == 1:
                                # bank0 complete: f/g activations + t1 early
                                sa = sact.tile([96, 768], bf16, name=f"sa_{d}")
                                nc.scalar.activation(sa[:, 0:192],
                                                     ps0[:, 0:192], AF.Sigmoid)
                                nc.scalar.activation(sa[:, 192:384],
                                                     ps0[:, 192:384], AF.Tanh)
                                t1 = tmp.tile([96, W], f32, name=f"t1_{d}")
                                nc.gpsimd.tensor_mul(t1[:], sa[:, 0:192],
                                                     ctl[d][p][:])

                        sf = sa[:, 0:192]
                        tg = sa[:, 192:384]
                        si = sa[:, 384:576]
                        so = sa[:, 576:768]
                        nc.scalar.activation(sa[:, 384:768], ps1[:, 0:384],
                                             AF.Sigmoid)
                        t2 = tmp.tile([96, W], bf16, name=f"t2_{d}")
                        nc.vector.tensor_mul(t2[:], si, tg)
                        nc.vector.tensor_add(ctl[d][pn][:], t1[:], t2[:])
                        tc2 = tmp.tile([96, W], bf16, name=f"tc2_{d}")
                        nc.scalar.activation(tc2[:], ctl[d][pn][:], AF.Tanh)
                        # h2 -> store (bf16), serves recurrence + projection
                        nc.vector.tensor_mul(hs[d][:, sl_out, 1:1 + W], so, tc2[:])

                # flush rows from the last two slots
                for k, ra in ((NCHUNK - 2, H - 2), (NCHUNK - 1, H - 1)):
                    passA(k, ra, H - 1 - ra)

            # second-half aggregate + combine (first half done in-scan).
            # s1 = (meanA+meanB)*n_half, s2 = (varA+meanA^2 + varB+meanB^2)*n_half
            nc.vector.bn_aggr(mv_b[:], statb[:, NCHUNK // 2:NCHUNK, :])
            n_half = float(HWTOT // 2)
            qa = const.tile([96, 1], f32, name="qa")
            nc.vector.tensor_mul(qa[:], mv_a[:, 0:1], mv_a[:, 0:1])
            nc.vector.tensor_add(qa[:], mv_a[:, 1:2], qa[:])
            qb = const.tile([96, 1], f32, name="qb")
            nc.vector.tensor_mul(qb[:], mv_b[:, 0:1], mv_b[:, 0:1])
            nc.vector.tensor_add(qb[:], mv_b[:, 1:2], qb[:])
            ssum = const.tile([96, 1], f32, name="ssum")
            nc.vector.tensor_add(ssum[:], mv_a[:, 0:1], mv_b[:, 0:1])
            qsum = const.tile([96, 1], f32, name="qsum")
            nc.vector.tensor_add(qsum[:], qa[:], qb[:])
            stats2 = const.tile([96, 2], f32, name="stats2")
            nc.vector.tensor_scalar_mul(stats2[:, 0:1], ssum[:], n_half)
            nc.vector.tensor_scalar_mul(stats2[:, 1:2], qsum[:], n_half)

            # --- AllReduce of [96,2] stats ----------------------------------
            ib = dram.tile([96, 2], f32, name="cc_in")
            ob = dram.tile([96, 2], f32, name="cc_out")
            nc.gpsimd.dma_start(ib[:], stats2[:])
            nc.gpsimd.collective_compute(
                "AllReduce",
                bass.mybir.AluOpType.add,
                replica_groups=[list(range(NCORES))],
                ins=[ib.opt()],
                outs=[ob.opt()],
            )
            gstats = const.tile([96, 2], f32, name="gstats")
            nc.gpsimd.dma_start(gstats[:], ob[:])

            # global mean / var -> a = gamma*rsqrt(var+eps), b = beta - mean*a
            inv_n = 1.0 / (NCORES * HWTOT)
            mu_g = const.tile([96, 1], f32, name="mu_g")
            nc.vector.tensor_scalar_mul(mu_g[:], gstats[:, 0:1], inv_n)
            ey2_g = const.tile([96, 1], f32, name="ey2_g")
            nc.vector.tensor_scalar_mul(ey2_g[:], gstats[:, 1:2], inv_n)
            musq = const.tile([96, 1], f32, name="musq")
            nc.vector.tensor_mul(musq[:], mu_g[:], mu_g[:])
            var_g = const.tile([96, 1], f32, name="var_g")
            nc.vector.tensor_sub(var_g[:], ey2_g[:], musq[:])
            sd = const.tile([96, 1], f32, name="sd")
            nc.scalar.activation(sd[:], var_g[:], AF.Sqrt, bias=eps_s[:])
            rs = const.tile([96, 1], f32, name="rs")
            nc.vector.reciprocal(rs[:], sd[:])
            a_s = const.tile([96, 1], f32, name="a_s")
            nc.vector.tensor_mul(a_s[:], gb_s[:, 0:1], rs[:])
            nma = const.tile([96, 1], f32, name="nma")
            nc.vector.tensor_mul(nma[:], mu_g[:], a_s[:])
            b_s = const.tile([96, 1], f32, name="b_s")
            nc.vector.tensor_sub(b_s[:], gb_s[:, 1:2], nma[:])

            # --- pass B: y already in the hs stores; relu(a*y+b)+x, out ----
            # x loads go on the sync DMA queue so they prefetch during the
            # AllReduce; output DMAs go on gpsimd.
            NCHB = H // 8  # 24 chunks x 8 rows (1536 cols)
            with (
                tc.tile_pool(name="fxin", bufs=8) as fxin,
                tc.tile_pool(name="frt", bufs=3) as frt,
            ):
                for k in range(NCHB):
                    if k < 12:
                        yv = hs[1][:, k * 8:k * 8 + 8, 1:1 + W]
                    else:
                        yv = hs[0][:, k * 8 + 1:k * 8 + 9, 1:1 + W]
                    # x for the skip-add re-read as bf16 (halves in-traffic)
                    xin = fxin.tile([96, 1536], bf16, name="xin")
                    nc.sync.dma_start(xin[:], xbf_d[:, k * 1536:(k + 1) * 1536])
                    rt = frt.tile([96, 8, 192], f32, name="rt", bufs=3)
                    nc.scalar.activation(rt[:], yv, AF.Relu,
                                         bias=b_s[:], scale=a_s[:])
                    rtf = rt[:].rearrange("p a b -> p (a b)")
                    # bf16 result tile: halves the output DMA traffic (host
                    # converts back to f32)
                    rtb = frt.tile([96, 1536], bf16, name="rtb", bufs=4)
                    nc.vector.tensor_add(rtb[:], rtf, xin[:])
                    co = k * 1536
                    nc.gpsimd.dma_start(out_d[:, co:co + 1536], rtb[:])

    nc.finalize()
    return nc


def kernel(x, w_f, b_f, w_b, b_b, w_proj, gamma, beta):
    _install_ntff_hook()
    from concourse.bass_utils import run_bass_kernel_spmd

    x = np.asarray(x, np.float32)
    wx, wh, wp = _prep_weights(
        np.asarray(w_f, np.float32), np.asarray(b_f, np.float32),
        np.asarray(w_b, np.float32), np.asarray(b_b, np.float32),
        np.asarray(w_proj, np.float32),
    )
    gb = np.stack([np.asarray(gamma, np.float32),
                   np.asarray(beta, np.float32)], axis=1)  # [96, 2]

    if 'nc' not in _cached:
        _cached['nc'] = _build_program()
    nc = _cached['nc']

    # wx built as [2, 97, 12, 96]; dram wants [97, 2, 12, 96] (partition-first)
    wx_in = np.ascontiguousarray(np.moveaxis(wx, 0, 1))
    wh_in = np.ascontiguousarray(np.moveaxis(wh, 0, 1))

    in_maps = []
    for b in range(NCORES):
        xb = np.ascontiguousarray(x[b].reshape(C, HWTOT))
        in_maps.append({
            "x": xb,
            "xbf": xb.astype(ml_dtypes.bfloat16),
            "wx": wx_in,
            "wh": wh_in,
            "wp": wp,
            "gamma_beta": gb,
        })
    res = run_bass_kernel_spmd(nc, in_maps, list(range(NCORES)))
    if res.exec_time_ns is not None:
        print(f"HW exec time: {res.exec_time_ns} ns")
    out = np.stack([res.results[b]["out"].reshape(C, H, W)
                    for b in range(NCORES)], axis=0)
    return out.astype(np.float32)


# revision 53
# speedup vs baseline: 1.1237x; 1.1237x over previous
"""Bidirectional ConvLSTM + 1x1 proj + BatchNorm + ReLU + skip, on 8 trn2 cores.

Sharding: data-parallel over batch (B=8 -> 1 batch element per core).
BatchNorm batch statistics are reduced across cores with a tiny AllReduce.

Per-core layout: channels on SBUF partitions (96), W on the free dim.
The H-recurrence runs forward and backward interleaved (192 slots x 2 dirs).
Each step's gate conv = 24 bf16 matmuls (4 gate blocks x 3 taps x {x,h}),
K=96(+1 bias row for x), M=96, N=192, accumulated in PSUM.
Hidden states are written (bf16) into a width-padded SBUF store that serves
as both the recurrence input (taps read the zero pad columns) and the
projection input.

Pass A (in-scan, lagged) computes y = proj(h_f, h_b) for completed rows,
feeds bn_stats, and writes y (bf16) back into the dead hs slots.  Pass B
after the stats AllReduce is then matmul-free: relu(a*y+b) + x -> out,
with x prefetched on the sync DMA queue during the AllReduce.
"""

import os
import sys
import types

import numpy as np
import ml_dtypes

B, C, H, W = 8, 96, 192, 192
HC = 96
EPS = 1e-5
NCORES = 8
WP = W + 2          # padded row width in the hidden-state store
NSLOT = H + 1       # store slots (one zero slot for the initial state)
HWTOT = H * W       # 36864
NCHUNK = H // 2     # projection chunks (2 rows each, N=384)

_cached = {}


def _install_ntff_hook():
    # Optional: lets BASS_TRACE=1 produce an NTFF profile under axon.
    if 'antenv.axon_hooks' in sys.modules:
        return
    try:
        import trn_agent_boot.trn_boot as tb
        hook = tb._ntff_profile_via_ctypes('/opt/axon/libaxon_pjrt.so')
        mod = types.ModuleType('antenv.axon_hooks')
        mod.get_axon_ntff_profile_hook = lambda: hook
        mod.set_axon_ntff_profile_hook = lambda h: None
        sys.modules['antenv.axon_hooks'] = mod
    except Exception:
        pass


def _prep_weights(w_f, b_f, w_b, b_b, w_proj):
    """Host-side weight packing into matmul-friendly lhsT layouts (bf16).

    Gate blocks (psum column order): 0=i, 1=f, 2=o, 3=g.
    Reference gate row ranges: i=0:96, f=96:192, g=192:288, o=288:384.
    wx[d]: [97, 12, 96]  (k=channel(+bias row 96), j=gb*3+tap, m=gate unit)
    wh[d]: [96, 12, 96]
    wp:    [96, 2, 96]   (k=hidden ch, d, m=out ch)
    """
    bf16 = ml_dtypes.bfloat16
    # gate block order: f, g, i, o  (bank0 = [f|g], bank1 = [i|o])
    gate_rows = [slice(96, 192), slice(192, 288), slice(0, 96), slice(288, 384)]
    wx = np.zeros((2, 97, 12, 96), np.float32)
    wh = np.zeros((2, 96, 12, 96), np.float32)
    for d, (w4, bias) in enumerate(((w_f, b_f), (w_b, b_b))):
        wmid = w4[:, :, 1, :]          # [384, 192, 3]
        for gb in range(4):
            rows = gate_rows[gb]
            for tap in range(3):
                j = gb * 3 + tap
                # x part: channels 0:96
                wx[d, 0:96, j, :] = wmid[rows, 0:96, tap].T
                # h part: channels 96:192
                wh[d, :, j, :] = wmid[rows, 96:192, tap].T
            # bias on the center tap's extra ones-row
            wx[d, 96, gb * 3 + 1, :] = bias[rows]
    wp = np.zeros((96, 2, 96), np.float32)
    wp[:, 0, :] = w_proj[:, 0:96].T
    wp[:, 1, :] = w_proj[:, 96:192].T
    return wx.astype(bf16), wh.astype(bf16), wp.astype(bf16)


def _build_program():
    import concourse.bass as bass
    import concourse.bacc as bacc
    import concourse.tile as tile
    from concourse import mybir

    f32 = mybir.dt.float32
    bf16 = mybir.dt.bfloat16
    AF = mybir.ActivationFunctionType

    nc = bacc.Bacc('TRN2', target_bir_lowering=False, debug=False,
                   num_devices=NCORES)

    x_d = nc.dram_tensor("x", [C, HWTOT], f32, kind="ExternalInput")
    xbf_d = nc.dram_tensor("xbf", [C, HWTOT], bf16, kind="ExternalInput")
    wx_d = nc.dram_tensor("wx", [97, 2, 12, 96], bf16, kind="ExternalInput")
    wh_d = nc.dram_tensor("wh", [96, 2, 12, 96], bf16, kind="ExternalInput")
    wp_d = nc.dram_tensor("wp", [96, 2, 96], bf16, kind="ExternalInput")
    gb_d = nc.dram_tensor("gamma_beta", [96, 2], f32, kind="ExternalInput")
    out_d = nc.dram_tensor("out", [C, HWTOT], bf16, kind="ExternalOutput")

    with tile.TileContext(nc) as tc:
        with (
            tc.tile_pool(name="const", bufs=1) as const,
            tc.tile_pool(name="dram", bufs=1, space="DRAM") as dram,
        ):
            # --- constants / persistent state -------------------------------
            gb_s = const.tile([96, 2], f32, name="gb_s")
            nc.gpsimd.dma_start(gb_s[:], gb_d[:])
            eps_s = const.tile([96, 1], f32, name="eps_s")
            nc.vector.memset(eps_s[:], EPS)

            # hidden-state stores, slot r width-padded. Only the pad columns
            # and the zero-state slot need zeroing (the scan writes the rest).
            hs = []
            for d in range(2):
                st = const.tile([96, NSLOT, WP], bf16, name=f"hs{d}")
                nc.vector.memset(st[:, :, 0:1], 0.0)
                nc.vector.memset(st[:, :, WP - 1:WP], 0.0)
                zslot = 0 if d == 0 else NSLOT - 1
                nc.vector.memset(st[:, zslot, :], 0.0)
                hs.append(st)

            statb = const.tile([96, NCHUNK, 6], f32, name="statb")
            mv_a = const.tile([96, 2], f32, name="mv_a")
            mv_b = const.tile([96, 2], f32, name="mv_b")

            # --- the scan ---------------------------------------------------
            with (
                tc.tile_pool(name="scanc", bufs=1) as scanc,
                tc.tile_pool(name="sact", bufs=4) as sact,
                tc.tile_pool(name="tmp", bufs=6) as tmp,
                tc.tile_pool(name="gpsum", bufs=2, space="PSUM") as gpsum,
            ):
                # scan-lifetime constants: freed before pass B so its DMA
                # pipeline can buffer deeper
                wx_s = scanc.tile([97, 2, 12, 96], bf16, name="wx_s")
                nc.gpsimd.dma_start(wx_s[:], wx_d[:])
                wh_s = scanc.tile([96, 2, 12, 96], bf16, name="wh_s")
                nc.gpsimd.dma_start(wh_s[:], wh_d[:])
                wp_s = scanc.tile([96, 2, 96], bf16, name="wp_s")
                nc.gpsimd.dma_start(wp_s[:], wp_d[:])

                # x-row tiles (97 rows: 96 ch + ones row), 4-deep rotation per
                # dir so the row DMA prefetches ahead of the matmul reads
                XRDEPTH = 4
                xr = [[None] * XRDEPTH, [None] * XRDEPTH]
                for d in range(2):
                    for p in range(XRDEPTH):
                        t = scanc.tile([97, WP], bf16, name=f"xr{d}{p}")
                        nc.vector.memset(t[0:96, 0:1], 0.0)
                        nc.vector.memset(t[0:96, WP - 1:WP], 0.0)
                        nc.vector.memset(t[96:97, :], 1.0)
                        xr[d][p] = t

                # cell state fp32, ping-pong per dir
                ctl = [[None, None], [None, None]]
                for d in range(2):
                    for p in range(2):
                        t = scanc.tile([96, W], f32, name=f"c{d}{p}")
                        nc.vector.memset(t[:], 0.0)
                        ctl[d][p] = t
                deferred_y = []

                def passA(k, ra, rb, defer_y=False):
                    """proj for rows (ra, rb) -> bn_stats; y (bf16) back into
                    the now-dead hs slots: y[ra] -> hs0 slot ra+1, y[rb] ->
                    hs1 slot rb.  defer_y postpones the y copies (pass B
                    input only) so the stats->AllReduce chain isn't delayed
                    behind them on the vector queue."""
                    psA = gpsum.tile([96, 512], f32, name="psA")
                    nc.tensor.matmul(psA[:, 0:192], wp_s[:, 0, :],
                                     hs[0][:, rb + 1, 1:1 + W],
                                     start=True, stop=False)
                    nc.tensor.matmul(psA[:, 0:192], wp_s[:, 1, :],
                                     hs[1][:, rb, 1:1 + W],
                                     start=False, stop=False)
                    nc.tensor.matmul(psA[:, 192:384], wp_s[:, 0, :],
                                     hs[0][:, ra + 1, 1:1 + W],
                                     start=False, stop=False)
                    nc.tensor.matmul(psA[:, 192:384], wp_s[:, 1, :],
                                     hs[1][:, ra, 1:1 + W],
                                     start=False, stop=True)
                    nc.vector.bn_stats(statb[:, k, :], psA[:, 0:384])
                    pairs = ((hs[1][:, rb, 1:1 + W], psA[:, 0:192]),
                             (hs[0][:, ra + 1, 1:1 + W], psA[:, 192:384]))
                    if defer_y:
                        deferred_y.extend(pairs)
                    else:
                        for dst, src in pairs:
                            nc.vector.tensor_copy(dst, src)

                # prime the x-row tiles for step 0
                for d in range(2):
                    r0 = 0 if d == 0 else H - 1
                    nc.gpsimd.dma_start(xr[d][0][0:96, 1:1 + W],
                                        xbf_d[:, r0 * W:(r0 + 1) * W])

                for t in range(H):
                    p, pn = t & 1, (t + 1) & 1
                    px = t % XRDEPTH

                    # prefetch next step's x rows before this step's t1 ops
                    # can block the gpsimd queue
                    if t + 1 < H:
                        pxn = (t + 1) % XRDEPTH
                        for d in range(2):
                            rn = t + 1 if d == 0 else H - 2 - t
                            nc.gpsimd.dma_start(xr[d][pxn][0:96, 1:1 + W],
                                                xbf_d[:, rn * W:(rn + 1) * W])

                    # fused pass A, lagged 2 slots so it never waits on the
                    # current slot's h2: project rows completed at slot t-2.
                    if t >= H // 2 + 2:
                        passA(t - H // 2 - 2, t - 2, H - 1 - (t - 2))
                    if t == H - 1:
                        # hoist the second-to-last chunk (needs only t<=190
                        # data) under the final step
                        passA(NCHUNK - 2, H - 2, 1, defer_y=True)
                    if t == 152:
                        # first-half stats aggregate, hidden in the scan
                        nc.vector.bn_aggr(mv_a[:], statb[:, 0:NCHUNK // 2, :])

                    for d in range(2):
                        r = t if d == 0 else H - 1 - t
                        # prev-h slot / out slot in the store
                        sl_in = r if d == 0 else r + 1
                        sl_out = r + 1 if d == 0 else r

                        # bank0 = [f|g], bank1 = [i|o]; bank0 first so that
                        # sig(f)/tanh(g)/t1 run under bank1's matmuls and the
                        # post-group chain is just sig(i,o)->t2->c2->tanh->h2.
                        ps0 = gpsum.tile([96, 512], f32, name=f"ps0_{d}", bufs=1)
                        ps1 = gpsum.tile([96, 512], f32, name=f"ps1_{d}")
                        hin = hs[d][:, sl_in, :]
                        banks = ((ps0, 0), (ps0, 192), (ps1, 0), (ps1, 192))
                        # all 12 x-matmuls first: they don't depend on h2, so
                        # the prior step's gate tail hides under them
                        for gbi, (pst, col) in enumerate(banks):
                            for tap in range(3):
                                j = gbi * 3 + tap
                                nc.tensor.matmul(
                                    pst[:, col:col + 192],
                                    wx_s[:, d, j, :],
                                    xr[d][px][:, tap:tap + W],
                                    start=(col == 0 and tap == 0),
                                    stop=False,
                                )
                        # then the 12 h-matmuls
                        for gbi, (pst, col) in enumerate(banks):
                            for tap in range(3):
                                j = gbi * 3 + tap
                                nc.tensor.matmul(
                                    pst[:, col:col + 192],
                                    wh_s[:, d, j, :],
                                    hin[:, tap:tap + W],
                                    start=False,
                                    stop=(col == 192 and tap == 2),
                                )

                            if gbi == 1:
                                # bank0 complete: f/g activations + t1 early
                                sa = sact.tile([96, 768], bf16, name=f"sa_{d}")
                                nc.scalar.activation(sa[:, 0:192],
                                                     ps0[:, 0:192], AF.Sigmoid)
                                nc.scalar.activation(sa[:, 192:384],
                                                     ps0[:, 192:384], AF.Tanh)
                                t1 = tmp.tile([96, W], f32, name=f"t1_{d}")
                                nc.gpsimd.tensor_mul(t1[:], sa[:, 0:192],
                                                     ctl[d][p][:])

                        sf = sa[:, 0:192]
                        tg = sa[:, 192:384]
                        si = sa[:, 384:576]
                        so = sa[:, 576:768]
                        nc.scalar.activation(sa[:, 384:768], ps1[:, 0:384],
                                             AF.Sigmoid)
                        t2 = tmp.tile([96, W], bf16, name=f"t2_{d}")
                        nc.vector.tensor_mul(t2[:], si, tg)
                        nc.vector.tensor_add(ctl[d][pn][:], t1[:], t2[:])
                        tc2 = tmp.tile([96, W], bf16, name=f"tc2_{d}")
                        nc.scalar.activation(tc2[:], ctl[d][pn][:], AF.Tanh)
                        # h2 -> store (bf16), serves recurrence + projection
                        nc.vector.tensor_mul(hs[d][:, sl_out, 1:1 + W], so, tc2[:])

                # flush the final chunk (needs the very last h2)
                passA(NCHUNK - 1, H - 1, 0, defer_y=True)

                # second-half aggregate + combine (first half done in-scan).
                # s1 = (meanA+meanB)*n_half, s2 = (varA+mA^2 + varB+mB^2)*n_half
                nc.vector.bn_aggr(mv_b[:], statb[:, NCHUNK // 2:NCHUNK, :])
                n_half = float(HWTOT // 2)
                qa = const.tile([96, 1], f32, name="qa")
                nc.vector.tensor_mul(qa[:], mv_a[:, 0:1], mv_a[:, 0:1])
                nc.vector.tensor_add(qa[:], mv_a[:, 1:2], qa[:])
                qb = const.tile([96, 1], f32, name="qb")
                nc.vector.tensor_mul(qb[:], mv_b[:, 0:1], mv_b[:, 0:1])
                nc.vector.tensor_add(qb[:], mv_b[:, 1:2], qb[:])
                ssum = const.tile([96, 1], f32, name="ssum")
                nc.vector.tensor_add(ssum[:], mv_a[:, 0:1], mv_b[:, 0:1])
                qsum = const.tile([96, 1], f32, name="qsum")
                nc.vector.tensor_add(qsum[:], qa[:], qb[:])
                stats2 = const.tile([96, 2], f32, name="stats2")
                nc.vector.tensor_scalar_mul(stats2[:, 0:1], ssum[:], n_half)
                nc.vector.tensor_scalar_mul(stats2[:, 1:2], qsum[:], n_half)

                # --- AllReduce of [96,2] stats ------------------------------
                ib = dram.tile([96, 2], f32, name="cc_in")
                ob = dram.tile([96, 2], f32, name="cc_out")
                nc.gpsimd.dma_start(ib[:], stats2[:])
                nc.gpsimd.collective_compute(
                    "AllReduce",
                    bass.mybir.AluOpType.add,
                    replica_groups=[list(range(NCORES))],
                    ins=[ib.opt()],
                    outs=[ob.opt()],
                )
                # y writebacks of the last two chunks, now safely behind the
                # collective trigger on the vector queue
                for dst, src in deferred_y:
                    nc.vector.tensor_copy(dst, src)

            gstats = const.tile([96, 2], f32, name="gstats")
            nc.gpsimd.dma_start(gstats[:], ob[:])

            # global mean / var -> a = gamma*rsqrt(var+eps), b = beta - mean*a
            inv_n = 1.0 / (NCORES * HWTOT)
            mu_g = const.tile([96, 1], f32, name="mu_g")
            nc.vector.tensor_scalar_mul(mu_g[:], gstats[:, 0:1], inv_n)
            ey2_g = const.tile([96, 1], f32, name="ey2_g")
            nc.vector.tensor_scalar_mul(ey2_g[:], gstats[:, 1:2], inv_n)
            musq = const.tile([96, 1], f32, name="musq")
            nc.vector.tensor_mul(musq[:], mu_g[:], mu_g[:])
            var_g = const.tile([96, 1], f32, name="var_g")
            nc.vector.tensor_sub(var_g[:], ey2_g[:], musq[:])
            sd = const.tile([96, 1], f32, name="sd")
            nc.scalar.activation(sd[:], var_g[:], AF.Sqrt, bias=eps_s[:])
            rs = const.tile([96, 1], f32, name="rs")
            nc.vector.reciprocal(rs[:], sd[:])
            a_s = const.tile([96, 1], f32, name="a_s")
            nc.vector.tensor_mul(a_s[:], gb_s[:, 0:1], rs[:])
            nma = const.tile([96, 1], f32, name="nma")
            nc.vector.tensor_mul(nma[:], mu_g[:], a_s[:])
            b_s = const.tile([96, 1], f32, name="b_s")
            nc.vector.tensor_sub(b_s[:], gb_s[:, 1:2], nma[:])

            # --- pass B: y already in the hs stores; relu(a*y+b)+x, out ----
            # x loads go on the sync DMA queue so they prefetch during the
            # AllReduce; output DMAs go on gpsimd.
            NCHB = H // 8  # 24 chunks x 8 rows (1536 cols)
            with (
                tc.tile_pool(name="fxin", bufs=8) as fxin,
                tc.tile_pool(name="frt", bufs=3) as frt,
            ):
                for k in range(NCHB):
                    if k < 12:
                        yv = hs[1][:, k * 8:k * 8 + 8, 1:1 + W]
                    else:
                        yv = hs[0][:, k * 8 + 1:k * 8 + 9, 1:1 + W]
                    # x for the skip-add re-read as bf16 (halves in-traffic)
                    xin = fxin.tile([96, 1536], bf16, name="xin")
                    nc.sync.dma_start(xin[:], xbf_d[:, k * 1536:(k + 1) * 1536])
                    rt = frt.tile([96, 8, 192], f32, name="rt", bufs=3)
                    nc.scalar.activation(rt[:], yv, AF.Relu,
                                         bias=b_s[:], scale=a_s[:])
                    rtf = rt[:].rearrange("p a b -> p (a b)")
                    # bf16 result tile: halves the output DMA traffic (host
                    # converts back to f32)
                    rtb = frt.tile([96, 1536], bf16, name="rtb", bufs=4)
                    nc.vector.tensor_add(rtb[:], rtf, xin[:])
                    co = k * 1536
                    nc.gpsimd.dma_start(out_d[:, co:co + 1536], rtb[:])

    nc.finalize()
    return nc


def kernel(x, w_f, b_f, w_b, b_b, w_proj, gamma, beta):
    _install_ntff_hook()
    from concourse.bass_utils import run_bass_kernel_spmd

    x = np.asarray(x, np.float32)
    wx, wh, wp = _prep_weights(
        np.asarray(w_f, np.float32), np.asarray(b_f, np.float32),
        np.asarray(w_b, np.float32), np.asarray(b_b, np.float32),
        np.asarray(w_proj, np.float32),
    )
    gb = np.stack([np.asarray(gamma, np.float32),
                   np.asarray(beta, np.float32)], axis=1)  # [96, 2]

    if 'nc' not in _cached:
        _cached['nc'] = _build_program()
    nc = _cached['nc']

    # wx built as [2, 97, 12, 96]; dram wants [97, 2, 12, 96] (partition-first)
    wx_in = np.ascontiguousarray(np.moveaxis(wx, 0, 1))
    wh_in = np.ascontiguousarray(np.moveaxis(wh, 0, 1))

    in_maps = []
    for b in range(NCORES):
        xb = np.ascontiguousarray(x[b].reshape(C, HWTOT))
        in_maps.append({
            "x": xb,
            "xbf": xb.astype(ml_dtypes.bfloat16),
            "wx": wx_in,
            "wh": wh_in,
            "wp": wp,
            "gamma_beta": gb,
        })
    res = run_bass_kernel_spmd(nc, in_maps, list(range(NCORES)))
    if res.exec_time_ns is not None:
        print(f"HW exec time: {res.exec_time_ns} ns")
    out = np.stack([res.results[b]["out"].reshape(C, H, W)
                    for b in range(NCORES)], axis=0)
    return out.astype(np.float32)
